# revision 15
# baseline (speedup 1.0000x reference)
"""Distributed Bass kernel for nn_AttentionCircuit (B=2,S=2048,D=2048,RANK=512,H=16).

Sharding: 8 cores = 2 batches x 4 group-positions. Core (b, g) computes
attention for head-group g (4 heads / 512 D-cols) of batch b over all S.

Two collectives per chunk, both off the critical path:
  - The gated low-rank projection t (A-stage) is rank-sharded: each core
    computes its own 128 rank rows (1/4 of the work) and an AllGather
    rebuilds the full [rank, SC] gated t. The AG for chunk t+2 is issued
    two chunks ahead so it always lands before B(t+2) needs it.
  - Instead of a ReduceScatter of [SC, D] W_O partials (2MiB/chunk,
    ~49us each, the last one an unoverlapped ~64us tail), each chunk's
    attention output ao is AllToAll'd across the group (each core sends
    the 128-query slices the peers own, 384KiB total) and every core
    computes its own 128 output rows against a full SBUF copy of W_O.
All matmul operands bf16 (1 cyc/row on PE, half the DMA bytes of fp32).

Per-core dataflow, streamed per 512-wide s-chunk:
  A(c): own 128 rank rows of t^T = read_own @ x_c^T, gated -> AllGather
  B(c): Q^T/K^T[own cols, chunk c], V[chunk c, own cols] from gathered t
  C(t=c): per head: scores^T = K^T.T Q^T (one pair ahead of PV) -> exp ->
     causal mask (block skip + static masks on diagonal) -> rowsum via
     ones-matmul of DVE quad-sums (lagged one pair) -> PV matmul ->
     normalize via fast fp32 1/Z + bcast matmul (1/0.81 folded into W_O)
  A2A(c): ao [dh, head, SC] -> peers' query slices; gather aoT_full
     [dh, 16 global heads, 128 own queries]
  D(c): own out rows = aoT_full.T @ W_O (full, contraction over all 2048)
Host reassembles: core (b, g) holds rows t*512+g*128..+128 of batch b.
"""
import sys
import numpy as np
import ml_dtypes

sys.path.insert(0, '/opt/trn_rl_repo')

import concourse.bass as bass  # noqa: E402
from concourse import bacc  # noqa: E402
import concourse.mybir as mybir  # noqa: E402
import concourse.tile as tile  # noqa: E402
from concourse.bass_utils import run_bass_kernel_spmd  # noqa: E402

B, S, D = 2, 2048, 2048
RANK = 512
NH = 16
HG = 4              # heads per core / group size
DHG = D // HG       # 512 cols per core
P = 128
DB = D // P         # 16 d-blocks
RB = RANK // P      # 4 rank-blocks (== own-col blocks)
SC = S // 4         # 512: s-chunk width == t-chunk width
NT = S // SC        # 4 chunks

F32 = mybir.dt.float32
F32R = mybir.dt.float32r
BF = mybir.dt.bfloat16
AF = mybir.ActivationFunctionType
ALU = mybir.AluOpType

EXP_SCALE = 1.0 / float(np.sqrt(P))
INV_KEEP2 = 1.0 / (0.9 * 0.9)
RGROUPS = [[0, 1, 2, 3], [4, 5, 6, 7]]

_CACHE = {}


def _r(ap):
    """[ (o p), f ] DRAM tensor -> [p, o, f] partition-tiled view."""
    return ap.rearrange("(o p) f -> p o f", p=P)


def _build():
    nc = bacc.Bacc("TRN2", target_bir_lowering=False, debug=False,
                   enable_asserts=False, num_devices=8)
    xT = nc.dram_tensor("xT", [D, S], BF, kind="ExternalInput").ap()
    gqT = nc.dram_tensor("gqT", [P, S], BF, kind="ExternalInput").ap()
    gkT = nc.dram_tensor("gkT", [P, S], BF, kind="ExternalInput").ap()
    gvT = nc.dram_tensor("gvT", [P, S], BF, kind="ExternalInput").ap()
    qk_readT = nc.dram_tensor("qk_readT", [D, P], BF, kind="ExternalInput").ap()
    v_readT = nc.dram_tensor("v_readT", [D, P], BF, kind="ExternalInput").ap()
    qk_w = nc.dram_tensor("qk_w", [RANK, DHG], BF, kind="ExternalInput").ap()
    v_w = nc.dram_tensor("v_w", [RANK, DHG], BF, kind="ExternalInput").ap()
    wo_full = nc.dram_tensor("wo_full", [D, D], BF, kind="ExternalInput").ap()
    gsel = nc.dram_tensor("gsel", [P, 2], F32, kind="ExternalInput").ap()
    out = nc.dram_tensor("out", [NT, P, D], BF, kind="ExternalOutput").ap()

    with tile.TileContext(nc) as tc:
        _body(tc, xT, gqT, gkT, gvT, qk_readT, v_readT, qk_w, v_w, wo_full,
              gsel, out)
    nc.compile()
    return nc


def _body(tc, xT, gqT, gkT, gvT, qk_readT, v_readT, qk_w, v_w, wo_full,
          gsel, out):
    nc = tc.nc
    import contextlib
    ctx = contextlib.ExitStack()
    with ctx:
        pool_main = ctx.enter_context(tc.tile_pool(name="main", bufs=1))
        pool_x = ctx.enter_context(tc.tile_pool(name="x", bufs=1))
        pool_g = ctx.enter_context(tc.tile_pool(name="g", bufs=1))
        pool_agin = ctx.enter_context(tc.tile_pool(name="agin", bufs=2))
        pool_tg = ctx.enter_context(tc.tile_pool(name="tg", bufs=2))
        pool_qt = ctx.enter_context(tc.tile_pool(name="qt", bufs=1))
        pool_ao = ctx.enter_context(tc.tile_pool(name="ao", bufs=1))
        pool_aot = ctx.enter_context(tc.tile_pool(name="aot", bufs=1))
        pool_osb = ctx.enter_context(tc.tile_pool(name="osb", bufs=1))
        pool_et = ctx.enter_context(tc.tile_pool(name="et", bufs=6))
        pool_ets = ctx.enter_context(tc.tile_pool(name="ets", bufs=3))
        pool_sm = ctx.enter_context(tc.tile_pool(name="sm", bufs=2))
        pool_dram = ctx.enter_context(tc.tile_pool(name="dramb", bufs=1,
                                                   space="DRAM"))
        psSC = ctx.enter_context(tc.tile_pool(name="psSC", bufs=5, space="PSUM"))
        psPV = ctx.enter_context(tc.tile_pool(name="psPV", bufs=2, space="PSUM"))
        psRS = ctx.enter_context(tc.tile_pool(name="psRS", bufs=1, space="PSUM"))

        # ---- long-lived tensors / constants
        KT_sb = pool_main.tile([P, HG, NT, SC], BF)   # K^T [dh, head, chunk, s]
        V_sb = pool_main.tile([P, DB, DHG], BF)       # V [s-block, own cols]
        wo_sb = pool_main.tile([P, DB, D], BF)        # full W_O (row-tiled)
        qr = pool_main.tile([P, DB, P], BF)           # own 128 rank cols
        vr = pool_main.tile([P, DB, P], BF)
        qw = pool_main.tile([P, RB, DHG], BF)
        vw = pool_main.tile([P, RB, DHG], BF)
        masks = pool_main.tile([P, HG, SC], BF)
        ones_r = pool_main.tile([P, 1], BF)
        onecol = pool_main.tile([1, P], F32)
        gsel_sb = pool_main.tile([P, 2], F32)
        nc.sync.dma_start(gsel_sb[:], gsel)

        # warm up the collective stream (absorbs the first-cc barrier/delay
        # while the big input DMAs run)
        warm_in = pool_dram.tile([P, 8], BF)
        warm_out = pool_dram.tile([RB, P, 8], BF)
        nc.gpsimd.collective_compute(
            "AllGather", ALU.bypass, ins=[warm_in.opt()],
            outs=[warm_out.opt()], replica_groups=RGROUPS)

        nc.sync.dma_start(qr[:], _r(qk_readT))
        nc.vector.memset(masks[:], 1.0)
        for o in range(HG):
            nc.gpsimd.affine_select(
                out=masks[:, o, :], in_=masks[:, o, :],
                compare_op=ALU.is_ge, fill=0.0, base=-P * o,
                pattern=[[1, SC]], channel_multiplier=-1)
        nc.vector.memset(ones_r[:], 1.0)
        nc.vector.memset(onecol[:], 1.0)

        ag_in_d = pool_dram.tile([NT, P, 3, SC], BF)
        ag_out_d = pool_dram.tile([NT, RB, P, 3, SC], BF)
        a2a_in = pool_dram.tile([NT, 8, P, HG, P], BF)
        a2a_out = pool_dram.tile([NT, 8, P, HG, P], BF)

        def dma_chunk_inputs(t):
            csl = slice(t * SC, (t + 1) * SC)
            xt = pool_x.tile([P, DB, SC], BF, tag="xt")
            nc.sync.dma_start(xt[:, :DB // 2, :], _r(xT)[:, :DB // 2, csl])
            nc.sync.dma_start(xt[:, DB // 2:, :], _r(xT)[:, DB // 2:, csl])
            gq = pool_g.tile([P, SC], BF, tag="gq")
            nc.sync.dma_start(gq[:], gqT[:, csl])
            gk = pool_g.tile([P, SC], BF, tag="gk")
            nc.sync.dma_start(gk[:], gkT[:, csl])
            gv = pool_g.tile([P, SC], BF, tag="gv")
            nc.sync.dma_start(gv[:], gvT[:, csl])
            return xt, gq, gk, gv

        def stage_a(t, ins):
            """Own 128 rank rows of gated t^T for s-chunk t -> AllGather."""
            xt, gq, gk, gv = ins
            agi = pool_agin.tile([P, 3, SC], BF, tag="agi")
            ps = psSC.tile([P, SC], F32, tag="sc")
            for db in range(DB):
                nc.tensor.matmul(ps[:], qr[:, db, :], xt[:, db, :],
                                 start=(db == 0), stop=(db == DB - 1))
            nc.vector.tensor_tensor(agi[:, 0, :], ps[:], gq[:], ALU.mult)
            nc.vector.tensor_tensor(agi[:, 1, :], ps[:], gk[:], ALU.mult)
            psv = psSC.tile([P, SC], F32, tag="sc")
            for db in range(DB):
                nc.tensor.matmul(psv[:], vr[:, db, :], xt[:, db, :],
                                 start=(db == 0), stop=(db == DB - 1))
            nc.vector.tensor_tensor(agi[:, 2, :], psv[:], gv[:], ALU.mult)
            nc.sync.dma_start(ag_in_d[t], agi[:])
            nc.gpsimd.collective_compute(
                "AllGather", ALU.bypass, ins=[ag_in_d[t].opt()],
                outs=[ag_out_d[t].opt()], replica_groups=RGROUPS)

        def tg_fetch(t):
            tg = pool_tg.tile([P, RB, 3, SC], BF, tag="tg")
            nc.sync.dma_start(
                tg[:], ag_out_d[t].rearrange("g p c f -> p g c f"))
            return tg

        def stage_b(t, tg, QT_sb):
            """Q^T/K^T [own cols, chunk t], V [chunk t, own cols]."""
            for db in range(RB):
                dsl = slice(db * P, (db + 1) * P)
                psq = psSC.tile([P, SC], F32, tag="sc")
                for rb in range(RB):
                    nc.tensor.matmul(psq[:], qw[:, rb, dsl], tg[:, rb, 0, :],
                                     start=(rb == 0), stop=(rb == RB - 1))
                nc.scalar.activation(QT_sb[:, db, :], psq[:], AF.Copy)
                psk = psSC.tile([P, SC], F32, tag="sc")
                for rb in range(RB):
                    nc.tensor.matmul(psk[:], qw[:, rb, dsl], tg[:, rb, 1, :],
                                     start=(rb == 0), stop=(rb == RB - 1))
                nc.scalar.activation(KT_sb[:, db, t, :], psk[:], AF.Copy)
            for sj in range(RB):
                sb = t * RB + sj
                ssl2 = slice(sj * P, (sj + 1) * P)
                psv = psSC.tile([P, DHG], F32, tag="sc")
                for rb in range(RB):
                    nc.tensor.matmul(psv[:], tg[:, rb, 2, ssl2], vw[:, rb, :],
                                     start=(rb == 0), stop=(rb == RB - 1))
                nc.scalar.activation(V_sb[:, sb, :], psv[:], AF.Copy)

        # ---- prologue: chunk-0/1 inputs, weights, A(0), A(1)
        ins0 = dma_chunk_inputs(0)
        nc.sync.dma_start(vr[:], _r(v_readT))
        nc.sync.dma_start(qw[:], _r(qk_w))
        nc.sync.dma_start(vw[:], _r(v_w))
        ins_next = dma_chunk_inputs(1)
        for wq in range(4):
            nc.sync.dma_start(wo_sb[:, wq * 4:(wq + 1) * 4, :],
                              _r(wo_full)[:, wq * 4:(wq + 1) * 4, :])
        stage_a(0, ins0)
        stage_a(1, ins_next)

        for t in range(NT):
            tg = tg_fetch(t)
            if t + 2 < NT:
                ins_next = dma_chunk_inputs(t + 2)
            QT = pool_qt.tile([P, HG, SC], BF, tag="qt")
            stage_b(t, tg, QT)

            # ---- C(t): attention for queries in chunk t, all own heads
            ao = pool_ao.tile([P, HG, SC], BF, tag="ao")
            npair = 2 * (t + 1)
            nquad = npair // 2

            def head_tail(h, pv, rs, e2args):
                """Finish head h: last quad rowsum, fast fp32 1/Z on DVE,
                f32r broadcast matmul, normalize. 1/0.81 is folded into W_O
                on the host."""
                e2, st, sp = e2args
                nc.tensor.matmul(rs[:], ones_r[:], e2[:], start=st, stop=sp)
                recip = pool_sm.tile([1, SC], F32, tag="recip")
                nc.vector.reciprocal_approx_fast(out=recip[:], in_=rs[:])
                rep = psSC.tile([P, SC], F32, tag="sc")
                nc.tensor.matmul(rep[:], onecol[:], recip[:],
                                 start=True, stop=True)
                nc.scalar.activation(ao[:, h, :], pv[:], AF.Copy)
                nc.vector.tensor_tensor(ao[:, h, :], ao[:, h, :], rep[:],
                                        ALU.mult)

            prev_tail = None
            for h in range(HG):
                hsl = slice(h * P, (h + 1) * P)
                pv = psPV.tile([P, SC], F32, tag="pv")
                rs = psRS.tile([1, SC], F32, tag="rs")
                pend_rs = []    # one-pair-lagged quad rowsum matmuls
                ets_hold = None

                def sc_pair(q):
                    """Emit scores+exp(+mask) for pair q; return et tiles."""
                    etps = []
                    for k in range(2):
                        jb = 2 * q + k
                        jc, jp = divmod(jb, RB)
                        sc = psSC.tile([P, SC], F32, tag="sc")
                        nc.tensor.matmul(
                            sc[:], KT_sb[:, h, jc, jp * P:(jp + 1) * P],
                            QT[:, h, :], start=True, stop=True)
                        etp = pool_et.tile([P, SC], BF, tag="et")
                        nc.scalar.activation(etp[:], sc[:], AF.Exp,
                                             scale=EXP_SCALE)
                        o = jb - 4 * t
                        if o >= 0:
                            nc.vector.tensor_tensor(etp[:], etp[:],
                                                    masks[:, o, :], ALU.mult)
                        etps.append(etp)
                    return etps

                # scores run one pair ahead of PV so the exp latency is
                # hidden behind the previous pair's PV matmuls
                etp_cur = sc_pair(0)
                for q in range(npair):
                    etp_next = sc_pair(q + 1) if q + 1 < npair else None
                    if pend_rs:
                        e2, st, sp = pend_rs.pop()
                        nc.tensor.matmul(rs[:], ones_r[:], e2[:],
                                         start=st, stop=sp)
                    for k in range(2):
                        jb = 2 * q + k
                        nc.tensor.matmul(pv[:], V_sb[:, jb, hsl],
                                         etp_cur[k][:],
                                         start=(q == 0 and k == 0),
                                         stop=(q == npair - 1 and k == 1))
                    if prev_tail is not None:
                        head_tail(*prev_tail)   # overlap prior head's tail
                        prev_tail = None
                    ets = pool_ets.tile([P, SC], BF, tag="ets")
                    nc.vector.tensor_tensor(ets[:], etp_cur[0][:],
                                            etp_cur[1][:], ALU.add)
                    if q % 2 == 0:
                        ets_hold = ets
                    else:
                        qd = q // 2
                        ets2 = pool_ets.tile([P, SC], BF, tag="ets2")
                        nc.vector.tensor_tensor(ets2[:], ets_hold[:], ets[:],
                                                ALU.add)
                        pend_rs.append((ets2, qd == 0, qd == nquad - 1))
                    etp_cur = etp_next
                prev_tail = (h, pv, rs, pend_rs.pop())
            head_tail(*prev_tail)   # last head: must finish before ao DMA
            prev_tail = None

            # ---- A2A(t): exchange ao query-slices across the group.
            # 4-core AllToAll is unsupported (mesh needs >4), so run it over
            # all 8 cores: each peer-slice is written to both candidate rank
            # slots (4G+j for G=0,1) and the receiver keeps the half from its
            # own group via a per-core 0/1 input (gsel) on the idle Pool
            # engine. Cross-group blocks carry the other batch's real ao, so
            # no NaN risk in the masked-out half.
            for g2 in range(HG):
                nc.sync.dma_start(a2a_in[t, g2],
                                  ao[:, :, g2 * P:(g2 + 1) * P])
                nc.sync.dma_start(a2a_in[t, HG + g2],
                                  ao[:, :, g2 * P:(g2 + 1) * P])
            nc.gpsimd.collective_compute(
                "AllToAll", ALU.bypass, ins=[a2a_in[t].opt()],
                outs=[a2a_out[t].opt()], replica_groups=[list(range(8))])
            t1 = pool_aot.tile([P, HG, HG, P], BF, tag="t1")
            t2 = pool_aot.tile([P, HG, HG, P], BF, tag="t2")
            nc.sync.dma_start(
                t1[:], a2a_out[t, :HG].rearrange("g p h q -> p g h q"))
            nc.sync.dma_start(
                t2[:], a2a_out[t, HG:].rearrange("g p h q -> p g h q"))
            nc.gpsimd.tensor_scalar(t1[:], t1[:], gsel_sb[:, 0:1], None,
                                    ALU.mult)
            nc.gpsimd.tensor_scalar(t2[:], t2[:], gsel_sb[:, 1:2], None,
                                    ALU.mult)
            nc.gpsimd.tensor_tensor(t1[:], t1[:], t2[:], ALU.add)
            aot = t1

            # ---- A(t+2) under the A2A window
            if t + 2 < NT:
                stage_a(t + 2, ins_next)

            # ---- D(t): own 128 out rows = aot.T @ W_O (full contraction)
            out_sb = pool_osb.tile([P, HG, SC], BF, tag="osb")
            for oc in range(4):
                psd = psSC.tile([P, SC], F32, tag="sc")
                for cb in range(DB):
                    nc.tensor.matmul(
                        psd[:], aot[:, cb // HG, cb % HG, :],
                        wo_sb[:, cb, oc * SC:(oc + 1) * SC],
                        start=(cb == 0), stop=(cb == DB - 1))
                nc.vector.tensor_copy(out_sb[:, oc, :], psd[:])
            nc.sync.dma_start(out[t],
                              out_sb[:].rearrange("p o f -> p (o f)"))


def _get_nc():
    if 'nc' not in _CACHE:
        _CACHE['nc'] = _build()
    return _CACHE['nc']


def _bf(a):
    return np.ascontiguousarray(np.asarray(a, np.float32)).astype(
        ml_dtypes.bfloat16)


def kernel(**inputs):
    x = np.asarray(inputs["x"], np.float32)
    g_Q = np.asarray(inputs["g_Q"], np.float32)
    g_K = np.asarray(inputs["g_K"], np.float32)
    g_V = np.asarray(inputs["g_V"], np.float32)
    qk_read = np.asarray(inputs["qk_read"], np.float32)
    qk_write = np.asarray(inputs["qk_write"], np.float32)
    v_read = np.asarray(inputs["v_read"], np.float32)
    v_write = np.asarray(inputs["v_write"], np.float32)
    W_O = np.asarray(inputs["W_O"], np.float32)

    nc = _get_nc()
    wo_b = _bf(W_O * INV_KEEP2)
    xTb = [_bf(x[b].T) for b in range(B)]
    gqTb = [_bf(g_Q[b].T) for b in range(B)]
    gkTb = [_bf(g_K[b].T) for b in range(B)]
    gvTb = [_bf(g_V[b].T) for b in range(B)]
    qk_readTg = [_bf(qk_read[g * P:(g + 1) * P, :].T) for g in range(4)]
    v_readTg = [_bf(v_read[g * P:(g + 1) * P, :].T) for g in range(4)]
    in_maps = []
    for c in range(8):
        b, g = divmod(c, 4)
        gsel_c = np.zeros((P, 2), np.float32)
        gsel_c[:, 0 if b == 0 else 1] = 1.0
        ssl = slice(g * DHG, (g + 1) * DHG)
        rsl = slice(g * P, (g + 1) * P)
        in_maps.append({
            "gsel": gsel_c,
            "xT": xTb[b],
            "gqT": np.ascontiguousarray(gqTb[b][rsl]),
            "gkT": np.ascontiguousarray(gkTb[b][rsl]),
            "gvT": np.ascontiguousarray(gvTb[b][rsl]),
            "qk_readT": qk_readTg[g],
            "v_readT": v_readTg[g],
            "qk_w": _bf(qk_write[:, ssl]),
            "v_w": _bf(v_write[:, ssl]),
            "wo_full": wo_b,
        })
    res = run_bass_kernel_spmd(nc, in_maps, core_ids=list(range(8)))
    _CACHE['last_results'] = res
    out = np.empty((B, S, D), np.float32)
    for c in range(8):
        b, g = divmod(c, 4)
        o = np.asarray(res.results[c]["out"], dtype=ml_dtypes.bfloat16)
        for t in range(NT):
            r0 = t * SC + g * P
            out[b, r0:r0 + P, :] = o[t].astype(np.float32)
    return out


# revision 16
# speedup vs baseline: 1.4348x; 1.4348x over previous
"""Distributed Bass kernel for nn_AttentionCircuit (B=2,S=2048,D=2048,RANK=512,H=16).

Sharding: 8 cores = 2 batches x 4 group-positions. Core (b, g) computes
attention for head-group g (4 heads / 512 D-cols) of batch b over all S.

Two collectives per chunk, both off the critical path:
  - The gated low-rank projection t (A-stage) is rank-sharded: each core
    computes its own 128 rank rows (1/4 of the work) and an AllGather
    rebuilds the full [rank, SC] gated t. The AG for chunk t+2 is issued
    two chunks ahead so it always lands before B(t+2) needs it.
  - Instead of a ReduceScatter of [SC, D] W_O partials (2MiB/chunk,
    ~49us each, the last one an unoverlapped ~64us tail), each chunk's
    attention output ao is AllToAll'd across the group (each core sends
    the 128-query slices the peers own, 384KiB total) and every core
    computes its own 128 output rows against a full SBUF copy of W_O.
All matmul operands bf16 (1 cyc/row on PE, half the DMA bytes of fp32).

Per-core dataflow, streamed per 512-wide s-chunk:
  A(c): own 128 rank rows of t^T = read_own @ x_c^T, gated -> AllGather
  B(c): Q^T/K^T[own cols, chunk c], V[chunk c, own cols] from gathered t
  C(t=c): per head: scores^T = K^T.T Q^T (one pair ahead of PV) -> exp ->
     causal mask (block skip + static masks on diagonal) -> rowsum via
     ones-matmul of DVE quad-sums (lagged one pair) -> PV matmul ->
     normalize via fast fp32 1/Z + bcast matmul (1/0.81 folded into W_O)
  A2A(c): ao [dh, head, SC] -> peers' query slices; gather aoT_full
     [dh, 16 global heads, 128 own queries]
  D(c): own out rows = aoT_full.T @ W_O (full, contraction over all 2048)
Host reassembles: core (b, g) holds rows t*512+g*128..+128 of batch b.
"""
import sys
import numpy as np
import ml_dtypes

sys.path.insert(0, '/opt/trn_rl_repo')

import concourse.bass as bass  # noqa: E402
from concourse import bacc  # noqa: E402
import concourse.mybir as mybir  # noqa: E402
import concourse.tile as tile  # noqa: E402
from concourse.bass_utils import run_bass_kernel_spmd  # noqa: E402

B, S, D = 2, 2048, 2048
RANK = 512
NH = 16
HG = 4              # heads per core / group size
DHG = D // HG       # 512 cols per core
P = 128
DB = D // P         # 16 d-blocks
RB = RANK // P      # 4 rank-blocks (== own-col blocks)
SC = S // 4         # 512: s-chunk width == t-chunk width
NT = S // SC        # 4 chunks

F32 = mybir.dt.float32
F32R = mybir.dt.float32r
BF = mybir.dt.bfloat16
AF = mybir.ActivationFunctionType
ALU = mybir.AluOpType

EXP_SCALE = 1.0 / float(np.sqrt(P))
INV_KEEP2 = 1.0 / (0.9 * 0.9)
RGROUPS = [[0, 1, 2, 3], [4, 5, 6, 7]]

_CACHE = {}


def _r(ap):
    """[ (o p), f ] DRAM tensor -> [p, o, f] partition-tiled view."""
    return ap.rearrange("(o p) f -> p o f", p=P)


def _build():
    nc = bacc.Bacc("TRN2", target_bir_lowering=False, debug=False,
                   enable_asserts=False, num_devices=8)
    xT = nc.dram_tensor("xT", [D, S], BF, kind="ExternalInput").ap()
    gqT = nc.dram_tensor("gqT", [P, S], BF, kind="ExternalInput").ap()
    gkT = nc.dram_tensor("gkT", [P, S], BF, kind="ExternalInput").ap()
    gvT = nc.dram_tensor("gvT", [P, S], BF, kind="ExternalInput").ap()
    qk_readT = nc.dram_tensor("qk_readT", [D, P], BF, kind="ExternalInput").ap()
    v_readT = nc.dram_tensor("v_readT", [D, P], BF, kind="ExternalInput").ap()
    qk_w = nc.dram_tensor("qk_w", [RANK, DHG], BF, kind="ExternalInput").ap()
    v_w = nc.dram_tensor("v_w", [RANK, DHG], BF, kind="ExternalInput").ap()
    wo_full = nc.dram_tensor("wo_full", [D, D], BF, kind="ExternalInput").ap()
    gsel = nc.dram_tensor("gsel", [P, 2], F32, kind="ExternalInput").ap()
    out = nc.dram_tensor("out", [NT, P, D], BF, kind="ExternalOutput").ap()

    with tile.TileContext(nc) as tc:
        _body(tc, xT, gqT, gkT, gvT, qk_readT, v_readT, qk_w, v_w, wo_full,
              gsel, out)
    nc.compile()
    return nc


def _body(tc, xT, gqT, gkT, gvT, qk_readT, v_readT, qk_w, v_w, wo_full,
          gsel, out):
    nc = tc.nc
    import contextlib
    ctx = contextlib.ExitStack()
    with ctx:
        pool_main = ctx.enter_context(tc.tile_pool(name="main", bufs=1))
        pool_x = ctx.enter_context(tc.tile_pool(name="x", bufs=1))
        pool_g = ctx.enter_context(tc.tile_pool(name="g", bufs=1))
        pool_agin = ctx.enter_context(tc.tile_pool(name="agin", bufs=2))
        pool_tg = ctx.enter_context(tc.tile_pool(name="tg", bufs=2))
        pool_qt = ctx.enter_context(tc.tile_pool(name="qt", bufs=1))
        pool_ao = ctx.enter_context(tc.tile_pool(name="ao", bufs=1))
        pool_aot = ctx.enter_context(tc.tile_pool(name="aot", bufs=1))
        pool_osb = ctx.enter_context(tc.tile_pool(name="osb", bufs=1))
        pool_et = ctx.enter_context(tc.tile_pool(name="et", bufs=6))
        pool_ets = ctx.enter_context(tc.tile_pool(name="ets", bufs=3))
        pool_sm = ctx.enter_context(tc.tile_pool(name="sm", bufs=2))
        pool_dram = ctx.enter_context(tc.tile_pool(name="dramb", bufs=1,
                                                   space="DRAM"))
        psSC = ctx.enter_context(tc.tile_pool(name="psSC", bufs=5, space="PSUM"))
        psPV = ctx.enter_context(tc.tile_pool(name="psPV", bufs=2, space="PSUM"))
        psRS = ctx.enter_context(tc.tile_pool(name="psRS", bufs=1, space="PSUM"))

        # ---- long-lived tensors / constants
        KT_sb = pool_main.tile([P, HG, NT, SC], BF)   # K^T [dh, head, chunk, s]
        V_sb = pool_main.tile([P, DB, DHG], BF)       # V [s-block, own cols]
        wo_sb = pool_main.tile([P, DB, D], BF)        # full W_O (row-tiled)
        qr = pool_main.tile([P, DB, P], BF)           # own 128 rank cols
        vr = pool_main.tile([P, DB, P], BF)
        qw = pool_main.tile([P, RB, DHG], BF)
        vw = pool_main.tile([P, RB, DHG], BF)
        masks = pool_main.tile([P, HG, SC], BF)
        ones_r = pool_main.tile([P, 1], BF)
        onecol = pool_main.tile([1, P], F32)
        gsel_sb = pool_main.tile([P, 2], F32)
        nc.scalar.dma_start(gsel_sb[:], gsel)

        # warm up the collective stream (absorbs the first-cc barrier/delay
        # while the big input DMAs run)
        warm_in = pool_dram.tile([P, 8], BF)
        warm_out = pool_dram.tile([RB, P, 8], BF)
        nc.gpsimd.collective_compute(
            "AllGather", ALU.bypass, ins=[warm_in.opt()],
            outs=[warm_out.opt()], replica_groups=RGROUPS)

        nc.scalar.dma_start(qr[:], _r(qk_readT))
        nc.vector.memset(masks[:], 1.0)
        for o in range(HG):
            nc.gpsimd.affine_select(
                out=masks[:, o, :], in_=masks[:, o, :],
                compare_op=ALU.is_ge, fill=0.0, base=-P * o,
                pattern=[[1, SC]], channel_multiplier=-1)
        nc.vector.memset(ones_r[:], 1.0)
        nc.vector.memset(onecol[:], 1.0)

        ag_in_d = pool_dram.tile([NT, P, 3, SC], BF)
        ag_out_d = pool_dram.tile([NT, RB, P, 3, SC], BF)
        a2a_in = pool_dram.tile([NT, 8, P, HG, P], BF)
        a2a_out = pool_dram.tile([NT, 8, P, HG, P], BF)

        def dma_chunk_inputs(t):
            csl = slice(t * SC, (t + 1) * SC)
            xt = pool_x.tile([P, DB, SC], BF, tag="xt")
            nc.scalar.dma_start(xt[:, :DB // 2, :], _r(xT)[:, :DB // 2, csl])
            nc.scalar.dma_start(xt[:, DB // 2:, :], _r(xT)[:, DB // 2:, csl])
            gq = pool_g.tile([P, SC], BF, tag="gq")
            nc.scalar.dma_start(gq[:], gqT[:, csl])
            gk = pool_g.tile([P, SC], BF, tag="gk")
            nc.scalar.dma_start(gk[:], gkT[:, csl])
            gv = pool_g.tile([P, SC], BF, tag="gv")
            nc.scalar.dma_start(gv[:], gvT[:, csl])
            return xt, gq, gk, gv

        def stage_a(t, ins):
            """Own 128 rank rows of gated t^T for s-chunk t -> AllGather."""
            xt, gq, gk, gv = ins
            agi = pool_agin.tile([P, 3, SC], BF, tag="agi")
            ps = psSC.tile([P, SC], F32, tag="sc")
            for db in range(DB):
                nc.tensor.matmul(ps[:], qr[:, db, :], xt[:, db, :],
                                 start=(db == 0), stop=(db == DB - 1))
            nc.vector.tensor_tensor(agi[:, 0, :], ps[:], gq[:], ALU.mult)
            nc.vector.tensor_tensor(agi[:, 1, :], ps[:], gk[:], ALU.mult)
            psv = psSC.tile([P, SC], F32, tag="sc")
            for db in range(DB):
                nc.tensor.matmul(psv[:], vr[:, db, :], xt[:, db, :],
                                 start=(db == 0), stop=(db == DB - 1))
            nc.vector.tensor_tensor(agi[:, 2, :], psv[:], gv[:], ALU.mult)
            nc.sync.dma_start(ag_in_d[t], agi[:])
            nc.gpsimd.collective_compute(
                "AllGather", ALU.bypass, ins=[ag_in_d[t].opt()],
                outs=[ag_out_d[t].opt()], replica_groups=RGROUPS)

        def tg_fetch(t):
            tg = pool_tg.tile([P, RB, 3, SC], BF, tag="tg")
            nc.sync.dma_start(
                tg[:], ag_out_d[t].rearrange("g p c f -> p g c f"))
            return tg

        def stage_b(t, tg, QT_sb):
            """Q^T/K^T [own cols, chunk t], V [chunk t, own cols]."""
            for db in range(RB):
                dsl = slice(db * P, (db + 1) * P)
                psq = psSC.tile([P, SC], F32, tag="sc")
                for rb in range(RB):
                    nc.tensor.matmul(psq[:], qw[:, rb, dsl], tg[:, rb, 0, :],
                                     start=(rb == 0), stop=(rb == RB - 1))
                nc.scalar.activation(QT_sb[:, db, :], psq[:], AF.Copy)
                psk = psSC.tile([P, SC], F32, tag="sc")
                for rb in range(RB):
                    nc.tensor.matmul(psk[:], qw[:, rb, dsl], tg[:, rb, 1, :],
                                     start=(rb == 0), stop=(rb == RB - 1))
                nc.scalar.activation(KT_sb[:, db, t, :], psk[:], AF.Copy)
            for sj in range(RB):
                sb = t * RB + sj
                ssl2 = slice(sj * P, (sj + 1) * P)
                psv = psSC.tile([P, DHG], F32, tag="sc")
                for rb in range(RB):
                    nc.tensor.matmul(psv[:], tg[:, rb, 2, ssl2], vw[:, rb, :],
                                     start=(rb == 0), stop=(rb == RB - 1))
                nc.scalar.activation(V_sb[:, sb, :], psv[:], AF.Copy)

        # ---- prologue: chunk-0/1 inputs, weights, A(0), A(1)
        ins0 = dma_chunk_inputs(0)
        nc.scalar.dma_start(vr[:], _r(v_readT))
        nc.scalar.dma_start(qw[:], _r(qk_w))
        nc.scalar.dma_start(vw[:], _r(v_w))
        ins_next = dma_chunk_inputs(1)
        for wq in range(4):
            nc.scalar.dma_start(wo_sb[:, wq * 4:(wq + 1) * 4, :],
                              _r(wo_full)[:, wq * 4:(wq + 1) * 4, :])
        stage_a(0, ins0)
        stage_a(1, ins_next)

        for t in range(NT):
            tg = tg_fetch(t)
            if t + 2 < NT:
                ins_next = dma_chunk_inputs(t + 2)
            QT = pool_qt.tile([P, HG, SC], BF, tag="qt")
            stage_b(t, tg, QT)

            # ---- C(t): attention for queries in chunk t, all own heads
            ao = pool_ao.tile([P, HG, SC], BF, tag="ao")
            npair = 2 * (t + 1)
            nquad = npair // 2

            def head_tail(h, pv, rs, e2args):
                """Finish head h: last quad rowsum, fast fp32 1/Z on DVE,
                f32r broadcast matmul, normalize. 1/0.81 is folded into W_O
                on the host."""
                e2, st, sp = e2args
                nc.tensor.matmul(rs[:], ones_r[:], e2[:], start=st, stop=sp)
                recip = pool_sm.tile([1, SC], F32, tag="recip")
                nc.vector.reciprocal_approx_fast(out=recip[:], in_=rs[:])
                rep = psSC.tile([P, SC], F32, tag="sc")
                nc.tensor.matmul(rep[:], onecol[:], recip[:],
                                 start=True, stop=True)
                nc.scalar.activation(ao[:, h, :], pv[:], AF.Copy)
                nc.vector.tensor_tensor(ao[:, h, :], ao[:, h, :], rep[:],
                                        ALU.mult)

            prev_tail = None
            for h in range(HG):
                hsl = slice(h * P, (h + 1) * P)
                pv = psPV.tile([P, SC], F32, tag="pv")
                rs = psRS.tile([1, SC], F32, tag="rs")
                pend_rs = []    # one-pair-lagged quad rowsum matmuls
                ets_hold = None

                def sc_pair(q):
                    """Emit scores+exp(+mask) for pair q; return et tiles."""
                    etps = []
                    for k in range(2):
                        jb = 2 * q + k
                        jc, jp = divmod(jb, RB)
                        sc = psSC.tile([P, SC], F32, tag="sc")
                        nc.tensor.matmul(
                            sc[:], KT_sb[:, h, jc, jp * P:(jp + 1) * P],
                            QT[:, h, :], start=True, stop=True)
                        etp = pool_et.tile([P, SC], BF, tag="et")
                        nc.scalar.activation(etp[:], sc[:], AF.Exp,
                                             scale=EXP_SCALE)
                        o = jb - 4 * t
                        if o >= 0:
                            nc.vector.tensor_tensor(etp[:], etp[:],
                                                    masks[:, o, :], ALU.mult)
                        etps.append(etp)
                    return etps

                # scores run one pair ahead of PV so the exp latency is
                # hidden behind the previous pair's PV matmuls
                etp_cur = sc_pair(0)
                for q in range(npair):
                    etp_next = sc_pair(q + 1) if q + 1 < npair else None
                    if pend_rs:
                        e2, st, sp = pend_rs.pop()
                        nc.tensor.matmul(rs[:], ones_r[:], e2[:],
                                         start=st, stop=sp)
                    for k in range(2):
                        jb = 2 * q + k
                        nc.tensor.matmul(pv[:], V_sb[:, jb, hsl],
                                         etp_cur[k][:],
                                         start=(q == 0 and k == 0),
                                         stop=(q == npair - 1 and k == 1))
                    if prev_tail is not None:
                        head_tail(*prev_tail)   # overlap prior head's tail
                        prev_tail = None
                    ets = pool_ets.tile([P, SC], BF, tag="ets")
                    nc.vector.tensor_tensor(ets[:], etp_cur[0][:],
                                            etp_cur[1][:], ALU.add)
                    if q % 2 == 0:
                        ets_hold = ets
                    else:
                        qd = q // 2
                        ets2 = pool_ets.tile([P, SC], BF, tag="ets2")
                        nc.vector.tensor_tensor(ets2[:], ets_hold[:], ets[:],
                                                ALU.add)
                        pend_rs.append((ets2, qd == 0, qd == nquad - 1))
                    etp_cur = etp_next
                prev_tail = (h, pv, rs, pend_rs.pop())
            head_tail(*prev_tail)   # last head: must finish before ao DMA
            prev_tail = None

            # ---- A2A(t): exchange ao query-slices across the group.
            # 4-core AllToAll is unsupported (mesh needs >4), so run it over
            # all 8 cores: each peer-slice is written to both candidate rank
            # slots (4G+j for G=0,1) and the receiver keeps the half from its
            # own group via a per-core 0/1 input (gsel) on the idle Pool
            # engine. Cross-group blocks carry the other batch's real ao, so
            # no NaN risk in the masked-out half.
            for g2 in range(HG):
                nc.sync.dma_start(a2a_in[t, g2],
                                  ao[:, :, g2 * P:(g2 + 1) * P])
                nc.sync.dma_start(a2a_in[t, HG + g2],
                                  ao[:, :, g2 * P:(g2 + 1) * P])
            nc.gpsimd.collective_compute(
                "AllToAll", ALU.bypass, ins=[a2a_in[t].opt()],
                outs=[a2a_out[t].opt()], replica_groups=[list(range(8))])
            t1 = pool_aot.tile([P, HG, HG, P], BF, tag="t1")
            t2 = pool_aot.tile([P, HG, HG, P], BF, tag="t2")
            nc.sync.dma_start(
                t1[:], a2a_out[t, :HG].rearrange("g p h q -> p g h q"))
            nc.sync.dma_start(
                t2[:], a2a_out[t, HG:].rearrange("g p h q -> p g h q"))
            # ---- A(t+2) under the A2A window
            if t + 2 < NT:
                stage_a(t + 2, ins_next)

            # keep the half of the A2A result from this core's own group
            # (emitted after stage_a so its gating ops aren't stuck behind
            # the t1/t2 wait on the DVE queue)
            nc.vector.tensor_scalar(t1[:], t1[:], gsel_sb[:, 0:1], None,
                                    ALU.mult)
            nc.vector.tensor_scalar(t2[:], t2[:], gsel_sb[:, 1:2], None,
                                    ALU.mult)
            nc.vector.tensor_tensor(t1[:], t1[:], t2[:], ALU.add)
            aot = t1

            # ---- D(t): own 128 out rows = aot.T @ W_O (full contraction)
            out_sb = pool_osb.tile([P, HG, SC], BF, tag="osb")
            for oc in range(4):
                psd = psSC.tile([P, SC], F32, tag="sc")
                for cb in range(DB):
                    nc.tensor.matmul(
                        psd[:], aot[:, cb // HG, cb % HG, :],
                        wo_sb[:, cb, oc * SC:(oc + 1) * SC],
                        start=(cb == 0), stop=(cb == DB - 1))
                nc.vector.tensor_copy(out_sb[:, oc, :], psd[:])
            nc.sync.dma_start(out[t],
                              out_sb[:].rearrange("p o f -> p (o f)"))


def _get_nc():
    if 'nc' not in _CACHE:
        _CACHE['nc'] = _build()
    return _CACHE['nc']


def _bf(a):
    return np.ascontiguousarray(np.asarray(a, np.float32)).astype(
        ml_dtypes.bfloat16)


def kernel(**inputs):
    x = np.asarray(inputs["x"], np.float32)
    g_Q = np.asarray(inputs["g_Q"], np.float32)
    g_K = np.asarray(inputs["g_K"], np.float32)
    g_V = np.asarray(inputs["g_V"], np.float32)
    qk_read = np.asarray(inputs["qk_read"], np.float32)
    qk_write = np.asarray(inputs["qk_write"], np.float32)
    v_read = np.asarray(inputs["v_read"], np.float32)
    v_write = np.asarray(inputs["v_write"], np.float32)
    W_O = np.asarray(inputs["W_O"], np.float32)

    nc = _get_nc()
    wo_b = _bf(W_O * INV_KEEP2)
    xTb = [_bf(x[b].T) for b in range(B)]
    gqTb = [_bf(g_Q[b].T) for b in range(B)]
    gkTb = [_bf(g_K[b].T) for b in range(B)]
    gvTb = [_bf(g_V[b].T) for b in range(B)]
    qk_readTg = [_bf(qk_read[g * P:(g + 1) * P, :].T) for g in range(4)]
    v_readTg = [_bf(v_read[g * P:(g + 1) * P, :].T) for g in range(4)]
    in_maps = []
    for c in range(8):
        b, g = divmod(c, 4)
        gsel_c = np.zeros((P, 2), np.float32)
        gsel_c[:, 0 if b == 0 else 1] = 1.0
        ssl = slice(g * DHG, (g + 1) * DHG)
        rsl = slice(g * P, (g + 1) * P)
        in_maps.append({
            "gsel": gsel_c,
            "xT": xTb[b],
            "gqT": np.ascontiguousarray(gqTb[b][rsl]),
            "gkT": np.ascontiguousarray(gkTb[b][rsl]),
            "gvT": np.ascontiguousarray(gvTb[b][rsl]),
            "qk_readT": qk_readTg[g],
            "v_readT": v_readTg[g],
            "qk_w": _bf(qk_write[:, ssl]),
            "v_w": _bf(v_write[:, ssl]),
            "wo_full": wo_b,
        })
    res = run_bass_kernel_spmd(nc, in_maps, core_ids=list(range(8)))
    _CACHE['last_results'] = res
    out = np.empty((B, S, D), np.float32)
    for c in range(8):
        b, g = divmod(c, 4)
        o = np.asarray(res.results[c]["out"], dtype=ml_dtypes.bfloat16)
        for t in range(NT):
            r0 = t * SC + g * P
            out[b, r0:r0 + P, :] = o[t].astype(np.float32)
    return out
